# revision 4
# baseline (speedup 1.0000x reference)
"""YIN pitch (nn_Pitch) Trainium2 Bass kernel.

Input  x  [8, 80000] f32  ->  output f0 [8, 996] f32.

Sharding: batch row b -> NeuronCore b (8 cores, no communication).

Per-core pipeline (single signal of 80000 samples):
  frames f = 0..996, window W=133, lags tau = 0..133, hop 80.
  d[f,tau] = sum_j (x[80f+j] - x[80f+j+tau])^2
           = E1[f] + E2[f,tau] - 2*C[f,tau]
  cmndf[f,tau] = d*tau / max(cumsum_{1..tau} d, 1e-8); threshold 0.2,
  first tau >= 20 below threshold -> f0 = 8000/tau else 0.

Layout: frames are placed one-per-partition in 8 blocks of 128 frames
(frame = 128*b + m).  Every stage is then partition-local:
  - XFR[b]  [128, 268]: frame windows (stride-80 strided DMA from DRAM)
  - C[f,tau] via DVE tensor_tensor_reduce (mult+sum in one instr per tau)
  - E-terms via squared prefix scan along the window
  - cmndf thresholding entirely as elementwise DVE ops; the divide is
    algebraically removed (d*tau < 0.2*max(cum,eps)).
"""

import numpy as np

import bass_rust
import concourse.bass as bass
import concourse.mybir as mybir
import concourse.tile as tile
from concourse.bass_utils import run_bass_kernel_spmd
from concourse.vector_clock import ScopedClock

# ---------------------------------------------------------------------------
# Wait-splitting post-pass: the walrus build in this container rejects
# instructions carrying more than ~2 semaphore waits ("Too many sync wait
# commands").  After Tile scheduling, move excess waits onto injected
# same-engine nops placed immediately before the instruction (same-engine
# program order makes this equivalent).
# ---------------------------------------------------------------------------

_WAIT_LIM = 1


def _split_excess_waits(nc):
    uid = 0
    for fn in nc.m.functions:
        for blk in fn.blocks:
            out = []
            changed = False
            for inst in blk.instructions:
                si = inst.sync_info
                waits = list(si.on_wait) if si is not None and si.on_wait else []
                if len(waits) > _WAIT_LIM:
                    changed = True
                    extra = waits[:-_WAIT_LIM]
                    si.on_wait = waits[-_WAIT_LIM:]
                    for j in range(0, len(extra), _WAIT_LIM):
                        nop = bass_rust.InstNoOp(
                            name=f"WSPLIT-{uid}", ins=[], outs=[]
                        )
                        uid += 1
                        nop.engine = inst.engine
                        nop.sync_info = bass_rust.SyncInfo(
                            on_wait=extra[j:j + _WAIT_LIM], on_update=[]
                        )
                        out.append(nop)
                out.append(inst)
            if changed:
                blk.instructions = out

# ---------------------------------------------------------------------------
# Problem constants (hardcoded from the model spec)
# ---------------------------------------------------------------------------
B = 8
N = 80000
SR = 8000
HOP = 80
TAU_MIN = 20
TAU_MAX = 133
W = 133            # integration window
FRAME_LEN = 266
N_FRAMES = 997     # 1 + (N - FRAME_LEN)//HOP
N_OUT = 996        # last frame dropped
THRESH = 0.2
EPS = 1e-8
BIG = 1.0e9

N_BLK = 8          # frame blocks of 128
FT = 268           # frame tile free size (266 used, +2 pad)

F32 = mybir.dt.float32
AluOp = mybir.AluOpType
Axis = mybir.AxisListType


def _ap(t, offset, pairs):
    return bass.AP(t, offset, pairs)


def _build_nc():
    nc = bass.Bass(trn_type="TRN2")
    x_d = nc.dram_tensor("x", [N], F32, kind="ExternalInput")
    f0_d = nc.dram_tensor("f0", [N_OUT], F32, kind="ExternalOutput")

    # constants baked into the NEFF
    tau_row = np.arange(1, TAU_MAX + 1, dtype=np.float32)           # [133]
    tauc_np = np.broadcast_to(tau_row, (128, W)).copy()
    taubig_np = (tauc_np + np.float32(BIG)).astype(np.float32)
    tauc_d = nc.inline_tensor(tauc_np, name="tauc")
    taubig_d = nc.inline_tensor(taubig_np, name="taubig")
    ident_d = nc.inline_tensor(np.eye(128, dtype=np.float32), name="ident")

    with tile.TileContext(nc) as tc:
        with (
            tc.tile_pool(name="persist", bufs=1) as pp,
            tc.tile_pool(name="work", bufs=2) as wp,
            tc.tile_pool(name="psum", bufs=1, space="PSUM") as psp,
        ):
            tauc = pp.tile([128, W], F32)
            nc.sync.dma_start(tauc[:], tauc_d[:])
            taubig = pp.tile([128, W], F32)
            nc.sync.dma_start(taubig[:], taubig_d[:])
            ident = pp.tile([128, 128], F32)
            nc.sync.dma_start(ident[:], ident_d[:])
            f0all = pp.tile([128, N_BLK], F32)
            nc.vector.memset(f0all[:], 0.0)

            for b in range(N_BLK):
                # frames 128b .. 128b+R-1 (block 7: only frames up to 996)
                R = 128 if b < N_BLK - 1 else N_FRAMES - 128 * (N_BLK - 1)

                xfr = wp.tile([128, FT], F32, tag="xfr")
                nc.sync.dma_start(
                    xfr[:R, :],
                    _ap(x_d, HOP * 128 * b, [[HOP, R], [1, FT]]),
                )

                # squared prefix sums for the energy terms
                sq = wp.tile([128, FRAME_LEN], F32, tag="sq")
                nc.scalar.square(sq[:R, :], xfr[:R, :FRAME_LEN])
                qq = wp.tile([128, FRAME_LEN], F32, tag="qq")
                nc.vector.tensor_tensor_scan(
                    qq[:R, :], sq[:R, :], sq[:R, :], 0.0, AluOp.add, AluOp.bypass
                )

                # autocorrelation C[f, tau], tau = 1..133
                c = wp.tile([128, TAU_MAX + 1], F32, tag="c")
                scr = wp.tile([128, W], F32, tag="scr")
                for tau in range(1, TAU_MAX + 1):
                    nc.vector.scalar_tensor_tensor(
                        out=scr[:R, :],
                        in0=xfr[:R, 0:W],
                        scalar=0.0,
                        in1=xfr[:R, tau:tau + W],
                        op0=AluOp.add,
                        op1=AluOp.mult,
                        accum_out=c[:R, tau:tau + 1],
                    )

                # d = E1 + E2 - 2C   (tau = 1..133)
                e2 = wp.tile([128, W], F32, tag="e2")
                nc.vector.tensor_sub(
                    e2[:R, :], qq[:R, W:FRAME_LEN], qq[:R, 0:W]
                )
                d = wp.tile([128, W], F32, tag="d")
                nc.vector.scalar_tensor_tensor(
                    out=d[:R, :],
                    in0=c[:R, 1:TAU_MAX + 1],
                    scalar=-2.0,
                    in1=e2[:R, :],
                    op0=AluOp.mult,
                    op1=AluOp.add,
                )
                nc.vector.tensor_scalar_add(d[:R, :], d[:R, :], qq[:R, W - 1:W])

                # cumulative sum of d over tau
                cum = wp.tile([128, W], F32, tag="cum")
                nc.vector.tensor_tensor_scan(
                    cum[:R, :], d[:R, :], d[:R, :], 0.0, AluOp.add, AluOp.bypass
                )

                # cand: d*tau < THRESH * max(cum, EPS)
                lhs = wp.tile([128, W], F32, tag="lhs")
                nc.vector.tensor_mul(lhs[:R, :], d[:R, :], tauc[:R, :])
                rhs = wp.tile([128, W], F32, tag="rhs")
                nc.vector.tensor_scalar(
                    out=rhs[:R, :],
                    in0=cum[:R, :],
                    scalar1=EPS,
                    scalar2=THRESH,
                    op0=AluOp.max,
                    op1=AluOp.mult,
                )
                cand = wp.tile([128, W], F32, tag="cand")
                nc.vector.tensor_tensor(
                    out=cand[:R, :], in0=lhs[:R, :], in1=rhs[:R, :], op=AluOp.is_lt
                )

                # first tau >= TAU_MIN with cand: min over (tau + BIG*(1-cand))
                v = wp.tile([128, W], F32, tag="v")
                nc.vector.scalar_tensor_tensor(
                    out=v[:R, :],
                    in0=cand[:R, :],
                    scalar=-BIG,
                    in1=taubig[:R, :],
                    op0=AluOp.mult,
                    op1=AluOp.add,
                )
                tmin = wp.tile([128, 1], F32, tag="tmin")
                nc.vector.tensor_reduce(
                    tmin[:R, :],
                    v[:R, TAU_MIN - 1:W],
                    axis=Axis.X,
                    op=AluOp.min,
                )

                # f0 = voiced ? SR / tau : 0
                voi = wp.tile([128, 1], F32, tag="voi")
                nc.vector.tensor_scalar(
                    out=voi[:R, :],
                    in0=tmin[:R, :],
                    scalar1=BIG * 0.5,
                    scalar2=None,
                    op0=AluOp.is_lt,
                )
                rec = wp.tile([128, 1], F32, tag="rec")
                nc.vector.reciprocal(rec[:R, :], tmin[:R, :])
                f0v = wp.tile([128, 1], F32, tag="f0v")
                nc.vector.tensor_scalar(
                    out=f0v[:R, :],
                    in0=rec[:R, :],
                    scalar1=float(SR),
                    scalar2=None,
                    op0=AluOp.mult,
                )
                nc.vector.tensor_mul(
                    f0all[:R, b:b + 1], f0v[:R, :], voi[:R, :]
                )

            # transpose [128, 8] -> [8, 128] so each block's f0 is contiguous
            f0t = psp.tile([N_BLK, 128], F32)
            nc.tensor.transpose(f0t[:], f0all[:, 0:N_BLK], ident[:])
            f0sb = pp.tile([N_BLK, 128], F32)
            nc.scalar.copy(f0sb[:], f0t[:])
            for b in range(N_BLK):
                cnt = 128 if b < N_BLK - 1 else N_OUT - 128 * (N_BLK - 1)
                nc.sync.dma_start(
                    _ap(f0_d, 128 * b, [[1, cnt]]),
                    f0sb[b:b + 1, 0:cnt],
                )

    _split_excess_waits(nc)
    return nc


_NC_CACHE = {}


def _get_nc():
    if "nc" not in _NC_CACHE:
        _NC_CACHE["nc"] = _build_nc()
    return _NC_CACHE["nc"]


def kernel(x: np.ndarray) -> np.ndarray:
    x = np.ascontiguousarray(np.asarray(x), dtype=np.float32)
    assert x.shape == (B, N), x.shape
    nc = _get_nc()
    in_maps = [{"x": x[i]} for i in range(B)]
    res = run_bass_kernel_spmd(nc, in_maps, core_ids=list(range(B)))
    out = np.stack([np.asarray(res.results[i]["f0"]).reshape(N_OUT) for i in range(B)])
    return out.astype(np.float32)
